# revision 3
# baseline (speedup 1.0000x reference)
"""GQA attention kernel for 8 TRN2 NeuronCores.

Problem: B=2, T=2048, DIM=2048, 16 Q-heads, 4 KV-heads, head_dim=128,
causal mask, RoPE variant y = rot(x) * (sin + cos).

Sharding: core = b * 4 + g  (b in 0..1 batch, g in 0..3 kv-group).
Each core computes 4 Q-heads + its KV head for one batch element, plus the
row-sharded slice of the output projection; the host sums the 4 partials per
batch (the "all-reduce") and adds bout.

Device-side algorithm per core (all fp32):
  - RoPE's pair-swap/negate is folded into Wq/Wk rows on the host, so on
    device RoPE is an elementwise multiply by mT = (sin+cos).T * 128^-0.25
    (the extra 128^-0.25 on both q and k realizes the 1/sqrt(128) score
    scale).
  - Projections computed transposed: qT/kT/vT[d, t] = W.T-tile.T @ xT-chunk.
  - v is PE-transposed back to natural V[s, d] layout.
  - Scores computed TRANSPOSED: ST[s, t] = kT-tile.T @ qT-chunk, so after
    exp() the result is already P^T, which is exactly the lhsT the PV matmul
    needs -- no per-tile transposes of P.
  - No max-subtraction in softmax (scores are O(10); exp cannot overflow;
    masked entries get -1e9 added -> exp == 0 exactly, matching the
    reference's where(mask, s, -1e9) + softmax).
  - Row sums via ones-vector matmul (M=1); reciprocal on DVE; broadcast of
    the reciprocal across partitions via a K=1 matmul; normalization fused
    into the PSUM->SBUF eviction of the PV accumulator.
"""

import numpy as np

import concourse.bacc as bacc
import concourse.bass as bass
import concourse.mybir as mybir
import concourse.tile as tile
from concourse import bass_utils
from concourse.masks import make_identity

F32 = mybir.dt.float32

DIM = 2048
T = 2048
B = 2
HD = 128          # head dim
HPC = 4           # q heads per core
QW = HPC * HD     # 512 q rows per core
NC_T = 4          # t-chunks of 512
TC = 512          # t-chunk width
NT = T // 128     # 16 tiles of 128 along t/s/c
NEG = -1e9


def build_kernel_nc():
    nc = bacc.Bacc("TRN2", target_bir_lowering=False, debug=False, num_devices=8)

    xT = nc.dram_tensor("xT", [DIM, T], F32, kind="ExternalInput").ap()
    wqT = nc.dram_tensor("wqT", [DIM, QW], F32, kind="ExternalInput").ap()
    wkT = nc.dram_tensor("wkT", [DIM, HD], F32, kind="ExternalInput").ap()
    wvT = nc.dram_tensor("wvT", [DIM, HD], F32, kind="ExternalInput").ap()
    woT = nc.dram_tensor("woT", [QW, DIM], F32, kind="ExternalInput").ap()
    mT = nc.dram_tensor("mT", [HD, T], F32, kind="ExternalInput").ap()
    maskT = nc.dram_tensor("maskT", [128, 4 * TC], F32, kind="ExternalInput").ap()
    out = nc.dram_tensor("out", [T, DIM], F32, kind="ExternalOutput").ap()

    with tile.TileContext(nc) as tc:
        emit(tc, nc, xT, wqT, wkT, wvT, woT, mT, maskT, out)

    nc.compile()
    return nc


def emit(tc, nc, xT, wqT, wkT, wvT, woT, mT, maskT, out):
    from contextlib import ExitStack

    ctx = ExitStack()
    singles = ctx.enter_context(tc.tile_pool(name="singles", bufs=1))
    qkv = ctx.enter_context(tc.tile_pool(name="qkv", bufs=1))
    xs = ctx.enter_context(tc.tile_pool(name="xs", bufs=4))
    pts = ctx.enter_context(tc.tile_pool(name="pts", bufs=3))
    sm = ctx.enter_context(tc.tile_pool(name="sm", bufs=2))
    outs = ctx.enter_context(tc.tile_pool(name="outs", bufs=3))
    vtmp = ctx.enter_context(tc.tile_pool(name="vtmp", bufs=2))
    ps = ctx.enter_context(tc.tile_pool(name="ps", bufs=8, space="PSUM"))

    # ---- constants / weights resident in SBUF ----
    mT_sb = singles.tile([HD, T], F32, tag="mT")
    nc.sync.dma_start(out=mT_sb, in_=mT)
    mask_sb = singles.tile([128, 4 * TC], F32, tag="mask")
    nc.sync.dma_start(out=mask_sb, in_=maskT)

    wq_sb = singles.tile([128, NT, QW], F32, tag="wq")
    nc.sync.dma_start(out=wq_sb, in_=wqT.rearrange("(a p) d -> p a d", p=128))
    wk_sb = singles.tile([128, NT, HD], F32, tag="wk")
    nc.sync.dma_start(out=wk_sb, in_=wkT.rearrange("(a p) d -> p a d", p=128))
    wv_sb = singles.tile([128, NT, HD], F32, tag="wv")
    nc.sync.dma_start(out=wv_sb, in_=wvT.rearrange("(a p) d -> p a d", p=128))
    wo_sb = singles.tile([128, HPC, DIM], F32, tag="wo")
    nc.sync.dma_start(out=wo_sb, in_=woT.rearrange("(a p) e -> p a e", p=128))

    ones_col = singles.tile([128, 1], F32, tag="ones_col")
    nc.vector.memset(ones_col, 1.0)
    ones_row = singles.tile([1, 128], F32, tag="ones_row")
    nc.vector.memset(ones_row, 1.0)
    ident = singles.tile([128, 128], F32, tag="ident")
    make_identity(nc, ident)

    # ---- persistent activations ----
    qT_sb = qkv.tile([128, HPC, T], F32, tag="qT")       # per head: [d, t]
    kT_sb = qkv.tile([128, T], F32, tag="kT")            # [d, s]
    v_sb = qkv.tile([128, NT, HD], F32, tag="v")         # per s-tile: [s, d]

    # =========== projections ===========
    for tci in range(NC_T):
        tsl = bass.ts(tci, TC)
        q_ps = [ps.tile([128, TC], F32, tag="ps", name=f"q_ps{h}") for h in range(HPC)]
        k_ps = ps.tile([128, TC], F32, tag="ps")
        v_ps = ps.tile([128, TC], F32, tag="ps")
        for c in range(NT):
            xch = xs.tile([128, TC], F32, tag="x")
            nc.sync.dma_start(out=xch, in_=xT[c * 128:(c + 1) * 128, tsl])
            st, sp = (c == 0), (c == NT - 1)
            for h in range(HPC):
                nc.tensor.matmul(q_ps[h], lhsT=wq_sb[:, c, h * HD:(h + 1) * HD],
                                 rhs=xch, start=st, stop=sp)
            nc.tensor.matmul(k_ps, lhsT=wk_sb[:, c, :], rhs=xch, start=st, stop=sp)
            nc.tensor.matmul(v_ps, lhsT=wv_sb[:, c, :], rhs=xch, start=st, stop=sp)
        # RoPE multiply (q, k); v: evict + transpose to natural layout
        for h in range(HPC):
            nc.vector.tensor_mul(qT_sb[:, h, tsl], q_ps[h], mT_sb[:, tsl])
        nc.vector.tensor_mul(kT_sb[:, tsl], k_ps, mT_sb[:, tsl])
        vt = vtmp.tile([128, TC], F32, tag="vt")
        nc.scalar.copy(vt, v_ps)
        for j in range(4):
            si = 4 * tci + j
            vn_ps = ps.tile([128, 128], F32, tag="ps")
            nc.tensor.transpose(vn_ps, vt[:, j * 128:(j + 1) * 128], ident)
            nc.scalar.copy(v_sb[:, si, :], vn_ps)

    # =========== attention + out-projection, per t-chunk ===========
    for tci in range(NC_T):
        tsl = bass.ts(tci, TC)
        nsi = 4 * tci + 4
        outT_sb = outs.tile([128, HPC, TC], F32, tag="outT")
        for h in range(HPC):
            pv_ps = ps.tile([128, TC], F32, tag="ps")
            rs_ps = ps.tile([1, TC], F32, tag="ps", padded_shape=[128, TC])
            for si in range(nsi):
                st_ps = ps.tile([128, TC], F32, tag="ps")
                nc.tensor.matmul(st_ps, lhsT=kT_sb[:, si * 128:(si + 1) * 128],
                                 rhs=qT_sb[:, h, tsl], start=True, stop=True)
                pt = pts.tile([128, TC], F32, tag="pt")
                oi = si - 4 * tci
                if oi >= 0:
                    nc.vector.tensor_add(st_ps, st_ps, mask_sb[:, oi * TC:(oi + 1) * TC])
                nc.scalar.activation(pt, st_ps, mybir.ActivationFunctionType.Exp)
                first, last = (si == 0), (si == nsi - 1)
                nc.tensor.matmul(rs_ps, lhsT=ones_col, rhs=pt, start=first, stop=last)
                nc.tensor.matmul(pv_ps, lhsT=v_sb[:, si, :], rhs=pt, start=first, stop=last)
            recip = sm.tile([1, TC], F32, tag="recip")
            nc.vector.reciprocal(recip, rs_ps)
            bc_ps = ps.tile([128, TC], F32, tag="ps")
            nc.tensor.matmul(bc_ps, lhsT=ones_row, rhs=recip, start=True, stop=True)
            bc_sb = sm.tile([128, TC], F32, tag="bc")
            nc.scalar.copy(bc_sb, bc_ps)
            nc.vector.tensor_mul(outT_sb[:, h, :], pv_ps, bc_sb)

        # out-projection for this t-chunk: partial[t, e] += outT_h.T @ woT_h
        for tt in range(4):
            t0 = tci * TC + tt * 128
            for ec in range(4):
                po_ps = ps.tile([128, TC], F32, tag="ps")
                for h in range(HPC):
                    nc.tensor.matmul(po_ps,
                                     lhsT=outT_sb[:, h, tt * 128:(tt + 1) * 128],
                                     rhs=wo_sb[:, h, ec * TC:(ec + 1) * TC],
                                     start=(h == 0), stop=(h == HPC - 1))
                ev = outs.tile([128, TC], F32, tag="ev")
                nc.scalar.copy(ev, po_ps)
                nc.sync.dma_start(out=out[t0:t0 + 128, ec * TC:(ec + 1) * TC], in_=ev)

    ctx.close()


# ---------------- host-side wrapper ----------------

_NC_CACHE = None


def _get_nc():
    global _NC_CACHE
    if _NC_CACHE is None:
        _NC_CACHE = build_kernel_nc()
    return _NC_CACHE


def _host_inputs(x, cos, sin, Wq, Wk, Wv, Wout):
    m = ((sin + cos) * np.float32(128.0 ** -0.25)).T  # [128, T]
    m = np.ascontiguousarray(m, dtype=np.float32)

    def rope_fold(W):
        Wr = np.empty_like(W)
        Wr[0::2] = -W[1::2]
        Wr[1::2] = W[0::2]
        return Wr

    Wq_r = rope_fold(np.asarray(Wq, dtype=np.float32))
    Wk_r = rope_fold(np.asarray(Wk, dtype=np.float32))

    # diagonal-band masks in ST layout: block oi: [s, t] valid iff t >= s + 128*oi
    s = np.arange(128)[:, None]
    t = np.arange(TC)[None, :]
    mask = np.concatenate(
        [np.where(t >= s + 128 * oi, 0.0, NEG).astype(np.float32) for oi in range(4)],
        axis=1)
    mask = np.ascontiguousarray(mask)

    maps = []
    for core in range(8):
        b, g = core // 4, core % 4
        maps.append({
            "xT": np.ascontiguousarray(x[b].T, dtype=np.float32),
            "wqT": np.ascontiguousarray(Wq_r[g * QW:(g + 1) * QW].T),
            "wkT": np.ascontiguousarray(Wk_r[g * HD:(g + 1) * HD].T),
            "wvT": np.ascontiguousarray(np.asarray(Wv, np.float32)[g * HD:(g + 1) * HD].T),
            "woT": np.ascontiguousarray(np.asarray(Wout, np.float32)[:, g * QW:(g + 1) * QW].T),
            "mT": m,
            "maskT": mask,
        })
    return maps


def kernel(x, cos, sin, mask, Wq, Wk, Wv, Wout, bout, _trace=False):
    nc = _get_nc()
    in_maps = _host_inputs(np.asarray(x, np.float32), np.asarray(cos, np.float32),
                           np.asarray(sin, np.float32), Wq, Wk, Wv, Wout)
    res = bass_utils.run_bass_kernel_spmd(nc, in_maps, core_ids=list(range(8)),
                                          trace=_trace)
    parts = [np.asarray(res.results[i]["out"]) for i in range(8)]
    bo = np.asarray(bout, np.float32)
    full = np.stack([parts[0] + parts[1] + parts[2] + parts[3] + bo,
                     parts[4] + parts[5] + parts[6] + parts[7] + bo])
    if _trace:
        return full.astype(np.float32), res
    return full.astype(np.float32)


# revision 4
# speedup vs baseline: 2.4283x; 2.4283x over previous
"""GQA attention kernel for 8 TRN2 NeuronCores.

Problem: B=2, T=2048, DIM=2048, 16 Q-heads, 4 KV-heads, head_dim=128,
causal mask, RoPE variant y = rot(x) * (sin + cos).

Sharding: core = b * 4 + g  (b in 0..1 batch, g in 0..3 kv-group).
Each core computes 4 Q-heads + its KV head for one batch element, plus the
row-sharded slice of the output projection; the host sums the 4 partials per
batch (the "all-reduce") and adds bout.

Device-side algorithm per core (all fp32):
  - RoPE's pair-swap/negate is folded into Wq/Wk rows on the host, so on
    device RoPE is an elementwise multiply by mT = (sin+cos).T * 128^-0.25
    (the extra 128^-0.25 on both q and k realizes the 1/sqrt(128) score
    scale).
  - Projections computed transposed: qT/kT/vT[d, t] = W.T-tile.T @ xT-chunk.
  - v is PE-transposed back to natural V[s, d] layout.
  - Scores computed TRANSPOSED: ST[s, t] = kT-tile.T @ qT-chunk, so after
    exp() the result is already P^T, which is exactly the lhsT the PV matmul
    needs -- no per-tile transposes of P.
  - No max-subtraction in softmax (scores are O(10); exp cannot overflow;
    masked entries get -1e9 added -> exp == 0 exactly, matching the
    reference's where(mask, s, -1e9) + softmax).
  - Row sums via ones-vector matmul (M=1); reciprocal on DVE; broadcast of
    the reciprocal across partitions via a K=1 matmul; normalization fused
    into the PSUM->SBUF eviction of the PV accumulator.
"""

import ml_dtypes
import numpy as np

import concourse.bacc as bacc
import concourse.bass as bass
import concourse.mybir as mybir
import concourse.tile as tile
from concourse import bass_utils
from concourse.masks import make_identity

F32 = mybir.dt.float32
BF16 = mybir.dt.bfloat16

DIM = 2048
T = 2048
B = 2
HD = 128          # head dim
HPC = 4           # q heads per core
QW = HPC * HD     # 512 q rows per core
NC_T = 4          # t-chunks of 512
TC = 512          # t-chunk width
NT = T // 128     # 16 tiles of 128 along t/s/c
NEG = -1e9


def build_kernel_nc():
    nc = bacc.Bacc("TRN2", target_bir_lowering=False, debug=False, num_devices=8)

    xT = nc.dram_tensor("xT", [DIM, T], BF16, kind="ExternalInput").ap()
    wqT = nc.dram_tensor("wqT", [DIM, QW], BF16, kind="ExternalInput").ap()
    wkT = nc.dram_tensor("wkT", [DIM, HD], BF16, kind="ExternalInput").ap()
    wvT = nc.dram_tensor("wvT", [DIM, HD], BF16, kind="ExternalInput").ap()
    woT = nc.dram_tensor("woT", [QW, DIM], BF16, kind="ExternalInput").ap()
    mT = nc.dram_tensor("mT", [HD, T], F32, kind="ExternalInput").ap()
    maskT = nc.dram_tensor("maskT", [128, 4 * TC], F32, kind="ExternalInput").ap()
    out = nc.dram_tensor("out", [T, DIM], F32, kind="ExternalOutput").ap()

    with tile.TileContext(nc) as tc:
        emit(tc, nc, xT, wqT, wkT, wvT, woT, mT, maskT, out)

    nc.compile()
    return nc


def emit(tc, nc, xT, wqT, wkT, wvT, woT, mT, maskT, out):
    from contextlib import ExitStack

    ctx = ExitStack()
    singles = ctx.enter_context(tc.tile_pool(name="singles", bufs=1))
    qkv = ctx.enter_context(tc.tile_pool(name="qkv", bufs=1))
    xs = ctx.enter_context(tc.tile_pool(name="xs", bufs=4))
    pts = ctx.enter_context(tc.tile_pool(name="pts", bufs=3))
    sm = ctx.enter_context(tc.tile_pool(name="sm", bufs=2))
    outs = ctx.enter_context(tc.tile_pool(name="outs", bufs=3))
    vtmp = ctx.enter_context(tc.tile_pool(name="vtmp", bufs=2))
    ps = ctx.enter_context(tc.tile_pool(name="ps", bufs=8, space="PSUM"))

    # ---- constants / weights resident in SBUF ----
    mT_sb = singles.tile([HD, T], F32, tag="mT")
    nc.sync.dma_start(out=mT_sb, in_=mT)
    mask_sb = singles.tile([128, 4 * TC], F32, tag="mask")
    nc.sync.dma_start(out=mask_sb, in_=maskT)

    wq_sb = singles.tile([128, NT, QW], BF16, tag="wq")
    nc.sync.dma_start(out=wq_sb, in_=wqT.rearrange("(a p) d -> p a d", p=128))
    wk_sb = singles.tile([128, NT, HD], BF16, tag="wk")
    nc.sync.dma_start(out=wk_sb, in_=wkT.rearrange("(a p) d -> p a d", p=128))
    wv_sb = singles.tile([128, NT, HD], BF16, tag="wv")
    nc.sync.dma_start(out=wv_sb, in_=wvT.rearrange("(a p) d -> p a d", p=128))
    wo_sb = singles.tile([128, HPC, DIM], BF16, tag="wo")
    nc.sync.dma_start(out=wo_sb, in_=woT.rearrange("(a p) e -> p a e", p=128))

    ones_col = singles.tile([128, 1], BF16, tag="ones_col")
    nc.vector.memset(ones_col, 1.0)
    ones_row = singles.tile([1, 128], F32, tag="ones_row")
    nc.vector.memset(ones_row, 1.0)
    ident = singles.tile([128, 128], BF16, tag="ident")
    make_identity(nc, ident)

    # ---- persistent activations ----
    qT_sb = qkv.tile([128, HPC, T], BF16, tag="qT")       # per head: [d, t]
    kT_sb = qkv.tile([128, T], BF16, tag="kT")            # [d, s]
    v_sb = qkv.tile([128, NT, HD], BF16, tag="v")         # per s-tile: [s, d]

    # =========== projections ===========
    for tci in range(NC_T):
        tsl = bass.ts(tci, TC)
        q_ps = [ps.tile([128, TC], F32, tag="ps", name=f"q_ps{h}") for h in range(HPC)]
        k_ps = ps.tile([128, TC], F32, tag="ps")
        v_ps = ps.tile([128, TC], F32, tag="ps")
        for c in range(NT):
            xch = xs.tile([128, TC], BF16, tag="x")
            nc.sync.dma_start(out=xch, in_=xT[c * 128:(c + 1) * 128, tsl])
            st, sp = (c == 0), (c == NT - 1)
            for h in range(HPC):
                nc.tensor.matmul(q_ps[h], lhsT=wq_sb[:, c, h * HD:(h + 1) * HD],
                                 rhs=xch, start=st, stop=sp)
            nc.tensor.matmul(k_ps, lhsT=wk_sb[:, c, :], rhs=xch, start=st, stop=sp)
            nc.tensor.matmul(v_ps, lhsT=wv_sb[:, c, :], rhs=xch, start=st, stop=sp)
        # RoPE multiply (q, k); v: evict + transpose to natural layout
        for h in range(HPC):
            nc.vector.tensor_mul(qT_sb[:, h, tsl], q_ps[h], mT_sb[:, tsl])
        nc.vector.tensor_mul(kT_sb[:, tsl], k_ps, mT_sb[:, tsl])
        vt = vtmp.tile([128, TC], BF16, tag="vt")
        nc.scalar.copy(vt, v_ps)
        for j in range(4):
            si = 4 * tci + j
            vn_ps = ps.tile([128, 128], BF16, tag="ps")
            nc.tensor.transpose(vn_ps, vt[:, j * 128:(j + 1) * 128], ident)
            nc.scalar.copy(v_sb[:, si, :], vn_ps)

    # =========== attention + out-projection, per t-chunk ===========
    for tci in range(NC_T):
        tsl = bass.ts(tci, TC)
        nsi = 4 * tci + 4
        outT_sb = outs.tile([128, HPC, TC], BF16, tag="outT")
        for h in range(HPC):
            pv_ps = ps.tile([128, TC], F32, tag="ps")
            rs_ps = ps.tile([1, TC], F32, tag="ps", padded_shape=[128, TC])
            for si in range(nsi):
                st_ps = ps.tile([128, TC], F32, tag="ps")
                nc.tensor.matmul(st_ps, lhsT=kT_sb[:, si * 128:(si + 1) * 128],
                                 rhs=qT_sb[:, h, tsl], start=True, stop=True)
                pt = pts.tile([128, TC], BF16, tag="pt")
                oi = si - 4 * tci
                if oi >= 0:
                    nc.vector.tensor_add(st_ps, st_ps, mask_sb[:, oi * TC:(oi + 1) * TC])
                nc.scalar.activation(pt, st_ps, mybir.ActivationFunctionType.Exp)
                first, last = (si == 0), (si == nsi - 1)
                nc.tensor.matmul(rs_ps, lhsT=ones_col, rhs=pt, start=first, stop=last)
                nc.tensor.matmul(pv_ps, lhsT=v_sb[:, si, :], rhs=pt, start=first, stop=last)
            recip = sm.tile([1, TC], F32, tag="recip")
            nc.vector.reciprocal(recip, rs_ps)
            bc_ps = ps.tile([128, TC], F32, tag="ps")
            nc.tensor.matmul(bc_ps, lhsT=ones_row, rhs=recip, start=True, stop=True)
            bc_sb = sm.tile([128, TC], F32, tag="bc")
            nc.scalar.copy(bc_sb, bc_ps)
            nc.vector.tensor_mul(outT_sb[:, h, :], pv_ps, bc_sb)

        # out-projection for this t-chunk: partial[t, e] += outT_h.T @ woT_h
        for tt in range(4):
            t0 = tci * TC + tt * 128
            for ec in range(4):
                po_ps = ps.tile([128, TC], F32, tag="ps")
                for h in range(HPC):
                    nc.tensor.matmul(po_ps,
                                     lhsT=outT_sb[:, h, tt * 128:(tt + 1) * 128],
                                     rhs=wo_sb[:, h, ec * TC:(ec + 1) * TC],
                                     start=(h == 0), stop=(h == HPC - 1))
                ev = outs.tile([128, TC], F32, tag="ev")
                nc.scalar.copy(ev, po_ps)
                nc.sync.dma_start(out=out[t0:t0 + 128, ec * TC:(ec + 1) * TC], in_=ev)

    ctx.close()


# ---------------- host-side wrapper ----------------

_NC_CACHE = None


def _get_nc():
    global _NC_CACHE
    if _NC_CACHE is None:
        _NC_CACHE = build_kernel_nc()
    return _NC_CACHE


def _host_inputs(x, cos, sin, Wq, Wk, Wv, Wout):
    m = ((sin + cos) * np.float32(128.0 ** -0.25)).T  # [128, T]
    m = np.ascontiguousarray(m, dtype=np.float32)

    def rope_fold(W):
        Wr = np.empty_like(W)
        Wr[0::2] = -W[1::2]
        Wr[1::2] = W[0::2]
        return Wr

    Wq_r = rope_fold(np.asarray(Wq, dtype=np.float32))
    Wk_r = rope_fold(np.asarray(Wk, dtype=np.float32))

    # diagonal-band masks in ST layout: block oi: [s, t] valid iff t >= s + 128*oi
    s = np.arange(128)[:, None]
    t = np.arange(TC)[None, :]
    mask = np.concatenate(
        [np.where(t >= s + 128 * oi, 0.0, NEG).astype(np.float32) for oi in range(4)],
        axis=1)
    mask = np.ascontiguousarray(mask)

    maps = []
    for core in range(8):
        b, g = core // 4, core % 4
        maps.append({
            "xT": np.ascontiguousarray(x[b].T).astype(ml_dtypes.bfloat16),
            "wqT": np.ascontiguousarray(Wq_r[g * QW:(g + 1) * QW].T).astype(ml_dtypes.bfloat16),
            "wkT": np.ascontiguousarray(Wk_r[g * HD:(g + 1) * HD].T).astype(ml_dtypes.bfloat16),
            "wvT": np.ascontiguousarray(np.asarray(Wv, np.float32)[g * HD:(g + 1) * HD].T).astype(ml_dtypes.bfloat16),
            "woT": np.ascontiguousarray(np.asarray(Wout, np.float32)[:, g * QW:(g + 1) * QW].T).astype(ml_dtypes.bfloat16),
            "mT": m,
            "maskT": mask,
        })
    return maps


def kernel(x, cos, sin, mask, Wq, Wk, Wv, Wout, bout, _trace=False):
    nc = _get_nc()
    in_maps = _host_inputs(np.asarray(x, np.float32), np.asarray(cos, np.float32),
                           np.asarray(sin, np.float32), Wq, Wk, Wv, Wout)
    res = bass_utils.run_bass_kernel_spmd(nc, in_maps, core_ids=list(range(8)),
                                          trace=_trace)
    parts = [np.asarray(res.results[i]["out"]) for i in range(8)]
    bo = np.asarray(bout, np.float32)
    full = np.stack([parts[0] + parts[1] + parts[2] + parts[3] + bo,
                     parts[4] + parts[5] + parts[6] + parts[7] + bo])
    if _trace:
        return full.astype(np.float32), res
    return full.astype(np.float32)


# revision 5
# speedup vs baseline: 3.0094x; 1.2393x over previous
"""GQA attention kernel for 8 TRN2 NeuronCores.

Problem: B=2, T=2048, DIM=2048, 16 Q-heads, 4 KV-heads, head_dim=128,
causal mask, RoPE variant y = rot(x) * (sin + cos).

Sharding: core = b * 4 + g  (b in 0..1 batch, g in 0..3 kv-group).
Each core computes 4 Q-heads + its KV head for one batch element, plus the
row-sharded slice of the output projection; the host sums the 4 partials per
batch (the "all-reduce") and adds bout.

Device-side algorithm per core (all fp32):
  - RoPE's pair-swap/negate is folded into Wq/Wk rows on the host, so on
    device RoPE is an elementwise multiply by mT = (sin+cos).T * 128^-0.25
    (the extra 128^-0.25 on both q and k realizes the 1/sqrt(128) score
    scale).
  - Projections computed transposed: qT/kT/vT[d, t] = W.T-tile.T @ xT-chunk.
  - v is PE-transposed back to natural V[s, d] layout.
  - Scores computed TRANSPOSED: ST[s, t] = kT-tile.T @ qT-chunk, so after
    exp() the result is already P^T, which is exactly the lhsT the PV matmul
    needs -- no per-tile transposes of P.
  - No max-subtraction in softmax (scores are O(10); exp cannot overflow;
    masked entries get -1e9 added -> exp == 0 exactly, matching the
    reference's where(mask, s, -1e9) + softmax).
  - Row sums via ones-vector matmul (M=1); reciprocal on DVE; broadcast of
    the reciprocal across partitions via a K=1 matmul; normalization fused
    into the PSUM->SBUF eviction of the PV accumulator.
"""

import ml_dtypes
import numpy as np

import concourse.bacc as bacc
import concourse.bass as bass
import concourse.mybir as mybir
import concourse.tile as tile
from concourse import bass_utils
from concourse.masks import make_identity

F32 = mybir.dt.float32
BF16 = mybir.dt.bfloat16

DIM = 2048
T = 2048
B = 2
HD = 128          # head dim
HPC = 4           # q heads per core
QW = HPC * HD     # 512 q rows per core
NC_T = 4          # t-chunks of 512
TC = 512          # t-chunk width
NT = T // 128     # 16 tiles of 128 along t/s/c
NEG = -1e9


def build_kernel_nc():
    nc = bacc.Bacc("TRN2", target_bir_lowering=False, debug=False, num_devices=8)

    xT = nc.dram_tensor("xT", [DIM, T], BF16, kind="ExternalInput").ap()
    wqT = nc.dram_tensor("wqT", [DIM, QW], BF16, kind="ExternalInput").ap()
    wkT = nc.dram_tensor("wkT", [DIM, HD], BF16, kind="ExternalInput").ap()
    wvT = nc.dram_tensor("wvT", [DIM, HD], BF16, kind="ExternalInput").ap()
    woT = nc.dram_tensor("woT", [QW, DIM], BF16, kind="ExternalInput").ap()
    mT = nc.dram_tensor("mT", [HD, T], F32, kind="ExternalInput").ap()
    maskT = nc.dram_tensor("maskT", [128, 4 * TC], BF16, kind="ExternalInput").ap()
    out = nc.dram_tensor("out", [T, DIM], F32, kind="ExternalOutput").ap()

    with tile.TileContext(nc) as tc:
        emit(tc, nc, xT, wqT, wkT, wvT, woT, mT, maskT, out)

    nc.compile()
    return nc


def emit(tc, nc, xT, wqT, wkT, wvT, woT, mT, maskT, out):
    from contextlib import ExitStack

    ctx = ExitStack()
    singles = ctx.enter_context(tc.tile_pool(name="singles", bufs=1))
    qkv = ctx.enter_context(tc.tile_pool(name="qkv", bufs=1))
    xs = ctx.enter_context(tc.tile_pool(name="xs", bufs=4))
    pts = ctx.enter_context(tc.tile_pool(name="pts", bufs=3))
    sm = ctx.enter_context(tc.tile_pool(name="sm", bufs=2))
    outs = ctx.enter_context(tc.tile_pool(name="outs", bufs=3))
    vtmp = ctx.enter_context(tc.tile_pool(name="vtmp", bufs=2))
    ps = ctx.enter_context(tc.tile_pool(name="ps", bufs=8, space="PSUM"))

    # ---- constants / weights resident in SBUF ----
    mT_sb = singles.tile([HD, T], F32, tag="mT")
    nc.sync.dma_start(out=mT_sb, in_=mT)
    mask_sb = singles.tile([128, 4 * TC], BF16, tag="mask")
    nc.sync.dma_start(out=mask_sb, in_=maskT)

    wq_sb = singles.tile([128, NT, QW], BF16, tag="wq")
    nc.sync.dma_start(out=wq_sb, in_=wqT.rearrange("(a p) d -> p a d", p=128))
    wk_sb = singles.tile([128, NT, HD], BF16, tag="wk")
    nc.sync.dma_start(out=wk_sb, in_=wkT.rearrange("(a p) d -> p a d", p=128))
    wv_sb = singles.tile([128, NT, HD], BF16, tag="wv")
    nc.sync.dma_start(out=wv_sb, in_=wvT.rearrange("(a p) d -> p a d", p=128))
    wo_sb = singles.tile([128, HPC, DIM], BF16, tag="wo")
    nc.sync.dma_start(out=wo_sb, in_=woT.rearrange("(a p) e -> p a e", p=128))

    ones_col = singles.tile([128, 1], BF16, tag="ones_col")
    nc.vector.memset(ones_col, 1.0)
    ident = singles.tile([128, 128], BF16, tag="ident")
    make_identity(nc, ident)

    # ---- persistent activations ----
    qT_sb = qkv.tile([128, HPC, T], BF16, tag="qT")       # per head: [d, t]
    kT_sb = qkv.tile([128, T], BF16, tag="kT")            # [d, s]
    v_sb = qkv.tile([128, NT, HD], BF16, tag="v")         # per s-tile: [s, d]

    # =========== projections ===========
    for tci in range(NC_T):
        tsl = bass.ts(tci, TC)
        q_ps = [ps.tile([128, TC], F32, tag="ps", name=f"q_ps{h}") for h in range(HPC)]
        k_ps = ps.tile([128, TC], F32, tag="ps")
        v_ps = ps.tile([128, TC], F32, tag="ps")
        for c in range(NT):
            xch = xs.tile([128, TC], BF16, tag="x")
            nc.sync.dma_start(out=xch, in_=xT[c * 128:(c + 1) * 128, tsl])
            st, sp = (c == 0), (c == NT - 1)
            for h in range(HPC):
                nc.tensor.matmul(q_ps[h], lhsT=wq_sb[:, c, h * HD:(h + 1) * HD],
                                 rhs=xch, start=st, stop=sp)
            nc.tensor.matmul(k_ps, lhsT=wk_sb[:, c, :], rhs=xch, start=st, stop=sp)
            nc.tensor.matmul(v_ps, lhsT=wv_sb[:, c, :], rhs=xch, start=st, stop=sp)
        # RoPE multiply (q, k); v: evict + transpose to natural layout
        for h in range(HPC):
            nc.vector.tensor_mul(qT_sb[:, h, tsl], q_ps[h], mT_sb[:, tsl])
        nc.vector.tensor_mul(kT_sb[:, tsl], k_ps, mT_sb[:, tsl])
        vt = vtmp.tile([128, TC], BF16, tag="vt")
        nc.scalar.copy(vt, v_ps)
        for j in range(4):
            si = 4 * tci + j
            vn_ps = ps.tile([128, 128], BF16, tag="ps")
            nc.tensor.transpose(vn_ps, vt[:, j * 128:(j + 1) * 128], ident)
            nc.scalar.copy(v_sb[:, si, :], vn_ps)

    # =========== attention + out-projection, per t-chunk ===========
    for tci in range(NC_T):
        tsl = bass.ts(tci, TC)
        nsi = 4 * tci + 4
        outT_sb = outs.tile([128, HPC, TC], BF16, tag="outT")
        for h in range(HPC):
            pv_ps = ps.tile([128, TC], F32, tag="ps")
            rs_ps = ps.tile([1, TC], F32, tag="ps", padded_shape=[128, TC])
            for si in range(nsi):
                st_ps = ps.tile([128, TC], F32, tag="ps")
                nc.tensor.matmul(st_ps, lhsT=kT_sb[:, si * 128:(si + 1) * 128],
                                 rhs=qT_sb[:, h, tsl], start=True, stop=True)
                pt = pts.tile([128, TC], BF16, tag="pt")
                nc.scalar.activation(pt, st_ps, mybir.ActivationFunctionType.Exp)
                oi = si - 4 * tci
                if oi >= 0:
                    nc.vector.tensor_mul(pt, pt, mask_sb[:, oi * TC:(oi + 1) * TC])
                first, last = (si == 0), (si == nsi - 1)
                nc.tensor.matmul(rs_ps, lhsT=ones_col, rhs=pt, start=first, stop=last)
                nc.tensor.matmul(pv_ps, lhsT=v_sb[:, si, :], rhs=pt, start=first, stop=last)
            pvu = outs.tile([128, TC], F32, tag="pvu")
            nc.scalar.copy(pvu, pv_ps)
            recip = sm.tile([1, TC], F32, tag="recip")
            nc.vector.reciprocal(recip, rs_ps)
            bcb = sm.tile([128, TC], F32, tag="bc")
            nc.gpsimd.partition_broadcast(bcb, recip)
            nc.vector.tensor_mul(outT_sb[:, h, :], pvu, bcb)

        # out-projection for this t-chunk: partial[t, e] += outT_h.T @ woT_h
        for tt in range(4):
            t0 = tci * TC + tt * 128
            for ec in range(4):
                po_ps = ps.tile([128, TC], F32, tag="ps")
                for h in range(HPC):
                    nc.tensor.matmul(po_ps,
                                     lhsT=outT_sb[:, h, tt * 128:(tt + 1) * 128],
                                     rhs=wo_sb[:, h, ec * TC:(ec + 1) * TC],
                                     start=(h == 0), stop=(h == HPC - 1))
                ev = outs.tile([128, TC], F32, tag="ev")
                nc.scalar.copy(ev, po_ps)
                nc.sync.dma_start(out=out[t0:t0 + 128, ec * TC:(ec + 1) * TC], in_=ev)

    ctx.close()


# ---------------- host-side wrapper ----------------

_NC_CACHE = None


def _get_nc():
    global _NC_CACHE
    if _NC_CACHE is None:
        _NC_CACHE = build_kernel_nc()
    return _NC_CACHE


def _host_inputs(x, cos, sin, Wq, Wk, Wv, Wout):
    m = ((sin + cos) * np.float32(128.0 ** -0.25)).T  # [128, T]
    m = np.ascontiguousarray(m, dtype=np.float32)

    def rope_fold(W):
        Wr = np.empty_like(W)
        Wr[0::2] = -W[1::2]
        Wr[1::2] = W[0::2]
        return Wr

    Wq_r = rope_fold(np.asarray(Wq, dtype=np.float32))
    Wk_r = rope_fold(np.asarray(Wk, dtype=np.float32))

    # diagonal-band masks in ST layout: block oi: [s, t] valid iff t >= s + 128*oi
    s = np.arange(128)[:, None]
    t = np.arange(TC)[None, :]
    mask = np.concatenate(
        [np.where(t >= s + 128 * oi, 1.0, 0.0).astype(np.float32) for oi in range(4)],
        axis=1)
    mask = np.ascontiguousarray(mask).astype(ml_dtypes.bfloat16)

    maps = []
    for core in range(8):
        b, g = core // 4, core % 4
        maps.append({
            "xT": np.ascontiguousarray(x[b].T).astype(ml_dtypes.bfloat16),
            "wqT": np.ascontiguousarray(Wq_r[g * QW:(g + 1) * QW].T).astype(ml_dtypes.bfloat16),
            "wkT": np.ascontiguousarray(Wk_r[g * HD:(g + 1) * HD].T).astype(ml_dtypes.bfloat16),
            "wvT": np.ascontiguousarray(np.asarray(Wv, np.float32)[g * HD:(g + 1) * HD].T).astype(ml_dtypes.bfloat16),
            "woT": np.ascontiguousarray(np.asarray(Wout, np.float32)[:, g * QW:(g + 1) * QW].T).astype(ml_dtypes.bfloat16),
            "mT": m,
            "maskT": mask,
        })
    return maps


def kernel(x, cos, sin, mask, Wq, Wk, Wv, Wout, bout, _trace=False):
    nc = _get_nc()
    in_maps = _host_inputs(np.asarray(x, np.float32), np.asarray(cos, np.float32),
                           np.asarray(sin, np.float32), Wq, Wk, Wv, Wout)
    res = bass_utils.run_bass_kernel_spmd(nc, in_maps, core_ids=list(range(8)),
                                          trace=_trace)
    parts = [np.asarray(res.results[i]["out"]) for i in range(8)]
    bo = np.asarray(bout, np.float32)
    full = np.stack([parts[0] + parts[1] + parts[2] + parts[3] + bo,
                     parts[4] + parts[5] + parts[6] + parts[7] + bo])
    if _trace:
        return full.astype(np.float32), res
    return full.astype(np.float32)


# revision 6
# speedup vs baseline: 3.0403x; 1.0103x over previous
"""GQA attention kernel for 8 TRN2 NeuronCores.

Problem: B=2, T=2048, DIM=2048, 16 Q-heads, 4 KV-heads, head_dim=128,
causal mask, RoPE variant y = rot(x) * (sin + cos).

Sharding: core = b * 4 + g  (b in 0..1 batch, g in 0..3 kv-group).
Each core computes 4 Q-heads + its KV head for one batch element, plus the
row-sharded slice of the output projection; the host sums the 4 partials per
batch (the "all-reduce") and adds bout.

Device-side algorithm per core (all fp32):
  - RoPE's pair-swap/negate is folded into Wq/Wk rows on the host, so on
    device RoPE is an elementwise multiply by mT = (sin+cos).T * 128^-0.25
    (the extra 128^-0.25 on both q and k realizes the 1/sqrt(128) score
    scale).
  - Projections computed transposed: qT/kT/vT[d, t] = W.T-tile.T @ xT-chunk.
  - v is PE-transposed back to natural V[s, d] layout.
  - Scores computed TRANSPOSED: ST[s, t] = kT-tile.T @ qT-chunk, so after
    exp() the result is already P^T, which is exactly the lhsT the PV matmul
    needs -- no per-tile transposes of P.
  - No max-subtraction in softmax (scores are O(10); exp cannot overflow;
    masked entries get -1e9 added -> exp == 0 exactly, matching the
    reference's where(mask, s, -1e9) + softmax).
  - Row sums via ones-vector matmul (M=1); reciprocal on DVE; broadcast of
    the reciprocal across partitions via a K=1 matmul; normalization fused
    into the PSUM->SBUF eviction of the PV accumulator.
"""

import ml_dtypes
import numpy as np

import concourse.bacc as bacc
import concourse.bass as bass
import concourse.mybir as mybir
import concourse.tile as tile
from concourse import bass_utils

F32 = mybir.dt.float32
BF16 = mybir.dt.bfloat16

DIM = 2048
T = 2048
B = 2
HD = 128          # head dim
HPC = 4           # q heads per core
QW = HPC * HD     # 512 q rows per core
NC_T = 4          # t-chunks of 512
TC = 512          # t-chunk width
NT = T // 128     # 16 tiles of 128 along t/s/c
NEG = -1e9


def build_kernel_nc():
    nc = bacc.Bacc("TRN2", target_bir_lowering=False, debug=False, num_devices=8)

    xT = nc.dram_tensor("xT", [DIM, T], BF16, kind="ExternalInput").ap()
    wqT = nc.dram_tensor("wqT", [DIM, QW], BF16, kind="ExternalInput").ap()
    wkT = nc.dram_tensor("wkT", [DIM, HD], BF16, kind="ExternalInput").ap()
    wvT = nc.dram_tensor("wvT", [DIM, HD], BF16, kind="ExternalInput").ap()
    woT = nc.dram_tensor("woT", [QW, DIM], BF16, kind="ExternalInput").ap()
    mT = nc.dram_tensor("mT", [HD, T], F32, kind="ExternalInput").ap()
    maskT = nc.dram_tensor("maskT", [128, 4 * TC], BF16, kind="ExternalInput").ap()
    out = nc.dram_tensor("out", [T, DIM], BF16, kind="ExternalOutput").ap()

    with tile.TileContext(nc) as tc:
        emit(tc, nc, xT, wqT, wkT, wvT, woT, mT, maskT, out)

    nc.compile()
    return nc


def emit(tc, nc, xT, wqT, wkT, wvT, woT, mT, maskT, out):
    from contextlib import ExitStack

    ctx = ExitStack()
    singles = ctx.enter_context(tc.tile_pool(name="singles", bufs=1))
    qkv = ctx.enter_context(tc.tile_pool(name="qkv", bufs=1))
    xs = ctx.enter_context(tc.tile_pool(name="xs", bufs=4))
    pts = ctx.enter_context(tc.tile_pool(name="pts", bufs=3))
    sm = ctx.enter_context(tc.tile_pool(name="sm", bufs=2))
    outs = ctx.enter_context(tc.tile_pool(name="outs", bufs=3))
    vtmp = ctx.enter_context(tc.tile_pool(name="vtmp", bufs=2))
    ps = ctx.enter_context(tc.tile_pool(name="ps", bufs=8, space="PSUM"))

    # ---- constants / weights resident in SBUF ----
    mT_sb = singles.tile([HD, T], F32, tag="mT")
    nc.sync.dma_start(out=mT_sb, in_=mT)
    mask_sb = singles.tile([128, 4 * TC], BF16, tag="mask")
    nc.sync.dma_start(out=mask_sb, in_=maskT)

    wq_sb = singles.tile([128, NT, QW], BF16, tag="wq")
    nc.sync.dma_start(out=wq_sb, in_=wqT.rearrange("(a p) d -> p a d", p=128))
    wk_sb = singles.tile([128, NT, HD], BF16, tag="wk")
    nc.sync.dma_start(out=wk_sb, in_=wkT.rearrange("(a p) d -> p a d", p=128))
    wv_sb = singles.tile([128, NT, HD], BF16, tag="wv")
    nc.sync.dma_start(out=wv_sb, in_=wvT.rearrange("(a p) d -> p a d", p=128))
    wo_sb = singles.tile([128, HPC, DIM], BF16, tag="wo")
    nc.sync.dma_start(out=wo_sb, in_=woT.rearrange("(a p) e -> p a e", p=128))

    ones_col = singles.tile([128, 1], BF16, tag="ones_col")
    nc.vector.memset(ones_col, 1.0)

    # ---- persistent activations ----
    qT_sb = qkv.tile([128, HPC, T], BF16, tag="qT")       # per head: [d, t]
    kT_sb = qkv.tile([128, T], BF16, tag="kT")            # [d, s]
    v_sb = qkv.tile([128, NT, HD], BF16, tag="v")         # per s-tile: [s, d]

    # =========== projections ===========
    for tci in range(NC_T):
        tsl = bass.ts(tci, TC)
        q_ps = [ps.tile([128, TC], F32, tag="ps", name=f"q_ps{h}") for h in range(HPC)]
        k_ps = ps.tile([128, TC], F32, tag="ps")
        v_ps = ps.tile([128, TC], F32, tag="ps")
        for c in range(NT):
            xch = xs.tile([128, TC], BF16, tag="x")
            nc.sync.dma_start(out=xch, in_=xT[c * 128:(c + 1) * 128, tsl])
            st, sp = (c == 0), (c == NT - 1)
            for h in range(HPC):
                nc.tensor.matmul(q_ps[h], lhsT=wq_sb[:, c, h * HD:(h + 1) * HD],
                                 rhs=xch, start=st, stop=sp)
            nc.tensor.matmul(k_ps, lhsT=wk_sb[:, c, :], rhs=xch, start=st, stop=sp)
            nc.tensor.matmul(v_ps, lhsT=wv_sb[:, c, :], rhs=xch, start=st, stop=sp)
        # RoPE multiply (q, k); v: evict + transpose to natural layout
        for h in range(HPC):
            nc.vector.tensor_mul(qT_sb[:, h, tsl], q_ps[h], mT_sb[:, tsl])
        nc.vector.tensor_mul(kT_sb[:, tsl], k_ps, mT_sb[:, tsl])
        vt = vtmp.tile([128, TC], BF16, tag="vt")
        nc.scalar.copy(vt, v_ps)
        for j in range(4):
            si = 4 * tci + j
            nc.sync.dma_start_transpose(v_sb[:, si, :], vt[:, j * 128:(j + 1) * 128])

    # =========== attention + out-projection, per t-chunk ===========
    for tci in range(NC_T):
        tsl = bass.ts(tci, TC)
        nsi = 4 * tci + 4
        outT_sb = outs.tile([128, HPC, TC], BF16, tag="outT")
        for h in range(HPC):
            pv_ps = ps.tile([128, TC], F32, tag="ps")
            rs_ps = ps.tile([1, TC], F32, tag="ps", padded_shape=[128, TC])
            for si in range(nsi):
                oi = si - 4 * tci
                lo = max(oi, 0) * 128   # cols < lo are fully masked
                w = TC - lo
                t0 = tci * TC + lo
                st_ps = ps.tile([128, TC], F32, tag="ps")
                nc.tensor.matmul(st_ps[:, :w], lhsT=kT_sb[:, si * 128:(si + 1) * 128],
                                 rhs=qT_sb[:, h, t0:t0 + w], start=True, stop=True)
                pt = pts.tile([128, TC], BF16, tag="pt")
                nc.scalar.activation(pt[:, :w], st_ps[:, :w],
                                     mybir.ActivationFunctionType.Exp)
                if oi >= 0:
                    nc.vector.tensor_mul(pt[:, :w], pt[:, :w],
                                         mask_sb[:, oi * TC + lo:(oi + 1) * TC])
                first, last = (si == 0), (si == nsi - 1)
                nc.tensor.matmul(rs_ps[:, lo:], lhsT=ones_col, rhs=pt[:, :w],
                                 start=first, stop=last)
                nc.tensor.matmul(pv_ps[:, lo:], lhsT=v_sb[:, si, :], rhs=pt[:, :w],
                                 start=first, stop=last)
            pvu = outs.tile([128, TC], F32, tag="pvu")
            nc.scalar.copy(pvu, pv_ps)
            recip = sm.tile([1, TC], F32, tag="recip")
            nc.vector.reciprocal(recip, rs_ps)
            bcb = sm.tile([128, TC], F32, tag="bc")
            nc.gpsimd.partition_broadcast(bcb, recip)
            nc.vector.tensor_mul(outT_sb[:, h, :], pvu, bcb)

        # out-projection for this t-chunk: partial[t, e] += outT_h.T @ woT_h
        for tt in range(4):
            t0 = tci * TC + tt * 128
            for ec in range(4):
                po_ps = ps.tile([128, TC], F32, tag="ps")
                for h in range(HPC):
                    nc.tensor.matmul(po_ps,
                                     lhsT=outT_sb[:, h, tt * 128:(tt + 1) * 128],
                                     rhs=wo_sb[:, h, ec * TC:(ec + 1) * TC],
                                     start=(h == 0), stop=(h == HPC - 1))
                ev = outs.tile([128, TC], BF16, tag="ev")
                nc.scalar.copy(ev, po_ps)
                nc.sync.dma_start(out=out[t0:t0 + 128, ec * TC:(ec + 1) * TC], in_=ev)

    ctx.close()


# ---------------- host-side wrapper ----------------

_NC_CACHE = None


def _get_nc():
    global _NC_CACHE
    if _NC_CACHE is None:
        _NC_CACHE = build_kernel_nc()
    return _NC_CACHE


def _host_inputs(x, cos, sin, Wq, Wk, Wv, Wout):
    m = ((sin + cos) * np.float32(128.0 ** -0.25)).T  # [128, T]
    m = np.ascontiguousarray(m, dtype=np.float32)

    def rope_fold(W):
        Wr = np.empty_like(W)
        Wr[0::2] = -W[1::2]
        Wr[1::2] = W[0::2]
        return Wr

    Wq_r = rope_fold(np.asarray(Wq, dtype=np.float32))
    Wk_r = rope_fold(np.asarray(Wk, dtype=np.float32))

    # diagonal-band masks in ST layout: block oi: [s, t] valid iff t >= s + 128*oi
    s = np.arange(128)[:, None]
    t = np.arange(TC)[None, :]
    mask = np.concatenate(
        [np.where(t >= s + 128 * oi, 1.0, 0.0).astype(np.float32) for oi in range(4)],
        axis=1)
    mask = np.ascontiguousarray(mask).astype(ml_dtypes.bfloat16)

    maps = []
    for core in range(8):
        b, g = core // 4, core % 4
        maps.append({
            "xT": np.ascontiguousarray(x[b].T).astype(ml_dtypes.bfloat16),
            "wqT": np.ascontiguousarray(Wq_r[g * QW:(g + 1) * QW].T).astype(ml_dtypes.bfloat16),
            "wkT": np.ascontiguousarray(Wk_r[g * HD:(g + 1) * HD].T).astype(ml_dtypes.bfloat16),
            "wvT": np.ascontiguousarray(np.asarray(Wv, np.float32)[g * HD:(g + 1) * HD].T).astype(ml_dtypes.bfloat16),
            "woT": np.ascontiguousarray(np.asarray(Wout, np.float32)[:, g * QW:(g + 1) * QW].T).astype(ml_dtypes.bfloat16),
            "mT": m,
            "maskT": mask,
        })
    return maps


def kernel(x, cos, sin, mask, Wq, Wk, Wv, Wout, bout, _trace=False):
    nc = _get_nc()
    in_maps = _host_inputs(np.asarray(x, np.float32), np.asarray(cos, np.float32),
                           np.asarray(sin, np.float32), Wq, Wk, Wv, Wout)
    res = bass_utils.run_bass_kernel_spmd(nc, in_maps, core_ids=list(range(8)),
                                          trace=_trace)
    parts = [np.asarray(res.results[i]["out"]).astype(np.float32) for i in range(8)]
    bo = np.asarray(bout, np.float32)
    full = np.stack([parts[0] + parts[1] + parts[2] + parts[3] + bo,
                     parts[4] + parts[5] + parts[6] + parts[7] + bo])
    if _trace:
        return full.astype(np.float32), res
    return full.astype(np.float32)
